# revision 2
# baseline (speedup 1.0000x reference)
"""Causal MHA with RoPE on 8 Trainium2 NeuronCores.

Sharding: tensor-parallel over heads. Core c owns heads {2c, 2c+1} (a 128-wide
slice of the model dim). Each core computes Q/K/V projections for its heads,
full causal attention, and a partial o_proj; the host sums the 8 partial
outputs (the "all-reduce").

Device layouts (per core):
  x.T   [128 i-part, 8 i-tile, t]   bf16, via DMA xbar transpose of bf16 x
  Q.T/K.T [128 hd, t] f32r, hd = [head A dims (ev0..31, od0..31), head B ...]
  RoPE = cos-mul + (permutation matmul on PE) sin-mul + add, tables from host
  scores S.T [k, q] = K.T-slice.T @ Q.T-slice (K=64, heads row-packed on PE)
  softmax: exp on ACT (no max subtraction; scores are O(1)), causal mask via
  gpsimd affine_select on the diagonal strip, denominators via ones-columns
  appended to V in the PV matmul (rows of PV psum), reciprocal+mul on DVE.
  o_proj: O.T tiles (f32r) x Wo.T slice -> partial y, written bf16.
"""
import sys
sys.path.insert(0, '/opt/trn_rl_repo')

import numpy as np
import ml_dtypes

import concourse.bass as bass
from concourse import bacc
import concourse.mybir as mybir
import concourse.tile as tile
from concourse.bass_utils import run_bass_kernel_spmd

BFNP = ml_dtypes.bfloat16
F32 = mybir.dt.float32
F32R = mybir.dt.float32r
BF16 = mybir.dt.bfloat16
AF = mybir.ActivationFunctionType

B, S, D = 4, 2048, 1024
NCORES = 8
BS = B * S
ROPE_THETA = 10000.0

TRACE = False
LAST_RESULTS = None


def build_nc(nb=B):
    nc = bacc.Bacc()
    xb = nc.dram_tensor("xb", [BS, D], BF16, kind="ExternalInput")
    wqt = nc.dram_tensor("wqt", [D, 128], BF16, kind="ExternalInput")
    wkt = nc.dram_tensor("wkt", [D, 128], BF16, kind="ExternalInput")
    wvt = nc.dram_tensor("wvt", [D, 128], BF16, kind="ExternalInput")
    wot = nc.dram_tensor("wot", [128, D], F32, kind="ExternalInput")
    cos2 = nc.dram_tensor("cos2", [128, S], F32, kind="ExternalInput")
    sins2 = nc.dram_tensor("sins2", [128, S], F32, kind="ExternalInput")
    pswap = nc.dram_tensor("pswap", [128, 128], F32, kind="ExternalInput")
    y = nc.dram_tensor("y", [BS, D], BF16, kind="ExternalOutput")

    with tile.TileContext(nc) as tc:
        with tc.tile_pool(name="const", bufs=1) as constp, \
             tc.tile_pool(name="xt", bufs=2) as xtp, \
             tc.tile_pool(name="qk", bufs=2) as qkp, \
             tc.tile_pool(name="vsb", bufs=2) as vsp, \
             tc.tile_pool(name="rope", bufs=3) as ropep, \
             tc.tile_pool(name="ptile", bufs=4) as pp, \
             tc.tile_pool(name="otp", bufs=2) as otp, \
             tc.tile_pool(name="rc", bufs=3) as rcp, \
             tc.tile_pool(name="yout", bufs=3) as yop:
            wq_sb = constp.tile([128, 8, 128], BF16)
            nc.scalar.dma_start(out=wq_sb, in_=wqt.rearrange("(a p) f -> p a f", p=128))
            wk_sb = constp.tile([128, 8, 128], BF16)
            nc.scalar.dma_start(out=wk_sb, in_=wkt.rearrange("(a p) f -> p a f", p=128))
            wv_sb = constp.tile([128, 8, 128], BF16)
            nc.scalar.dma_start(out=wv_sb, in_=wvt.rearrange("(a p) f -> p a f", p=128))
            wot_sb = constp.tile([128, D], F32R)
            nc.scalar.dma_start(out=wot_sb, in_=wot[:, :].bitcast(F32R))
            cos_sb = constp.tile([128, S], F32)
            nc.scalar.dma_start(out=cos_sb, in_=cos2[:, :])
            sin_sb = constp.tile([128, S], F32)
            nc.scalar.dma_start(out=sin_sb, in_=sins2[:, :])
            psw_sb = constp.tile([128, 128], F32R)
            nc.scalar.dma_start(out=psw_sb, in_=pswap[:, :].bitcast(F32R))

            for b in range(nb):
                tb0 = b * S
                qtr = qkp.tile([128, S], F32R, tag="qtr")
                ktr = qkp.tile([128, S], F32R, tag="ktr")
                v_sb = vsp.tile([128, 16, 192], BF16, tag="v")
                nc.gpsimd.memset(v_sb[:, :, 64:128], 1.0)
                ot_sb = otp.tile([128, S], F32R, tag="ot")

                # ---- projections + rope, per 512-token chunk ----
                with tc.tile_pool(name=f"pj{b}", bufs=6, space="PSUM") as pjp, \
                     tc.tile_pool(name=f"sw{b}", bufs=2, space="PSUM") as swp:
                    for c4 in range(4):
                        t0 = 512 * c4
                        xt = xtp.tile([128, 8, 512], BF16, tag="xt")
                        nc.sync.dma_start_transpose(xt, xb[tb0 + t0:tb0 + t0 + 512, :])
                        qt_ps = pjp.tile([128, 512], F32, tag="pj")
                        kt_ps = pjp.tile([128, 512], F32, tag="pj")
                        vt_ps = pjp.tile([128, 512], F32, tag="pj")
                        for it in range(8):
                            nc.tensor.matmul(qt_ps, wq_sb[:, it, :], xt[:, it, :],
                                             start=(it == 0), stop=(it == 7))
                        for it in range(8):
                            nc.tensor.matmul(kt_ps, wk_sb[:, it, :], xt[:, it, :],
                                             start=(it == 0), stop=(it == 7))
                        for tt in range(4):
                            for it in range(8):
                                nc.tensor.matmul(vt_ps[:, 128 * tt:128 * tt + 128],
                                                 xt[:, it, 128 * tt:128 * tt + 128],
                                                 wv_sb[:, it, :],
                                                 start=(it == 0), stop=(it == 7))
                        # rope: dst = ps*cos + (pswap.T @ ps)*sins
                        for ps_t, dst in ((qt_ps, qtr), (kt_ps, ktr)):
                            sb_c = ropep.tile([128, 512], F32R, tag="rcopy")
                            nc.vector.tensor_copy(sb_c, ps_t)
                            sw_ps = swp.tile([128, 512], F32, tag="sw")
                            nc.tensor.matmul(sw_ps, psw_sb, sb_c, start=True, stop=True)
                            tmp = ropep.tile([128, 512], F32, tag="rtmp")
                            nc.vector.tensor_mul(tmp, sw_ps, sin_sb[:, t0:t0 + 512])
                            nc.vector.tensor_mul(dst[:, t0:t0 + 512], ps_t, cos_sb[:, t0:t0 + 512])
                            nc.vector.tensor_add(dst[:, t0:t0 + 512], dst[:, t0:t0 + 512], tmp)
                        # v: [t, hd] -> v_sb k-tiles [VA(64) | ones(64) | VB(64)]
                        vv = vt_ps.rearrange("p (t c) -> p t c", t=4)
                        nc.vector.tensor_copy(v_sb[:, 4 * c4:4 * c4 + 4, 0:64], vv[:, :, 0:64])
                        nc.vector.tensor_copy(v_sb[:, 4 * c4:4 * c4 + 4, 128:192], vv[:, :, 64:128])

                # ---- attention, per 512-query chunk ----
                with tc.tile_pool(name=f"st{b}", bufs=2, space="PSUM") as stp, \
                     tc.tile_pool(name=f"pv{b}", bufs=3, space="PSUM") as pvp:
                    for qc in range(4):
                        q0 = 512 * qc
                        npair = 2 * (qc + 1)
                        pv_ps = []
                        for _hh in range(2):
                            pv_one = pvp.tile([128, 512], F32, tag="pv")
                            pv_ps.append(pv_one)
                        for kp in range(npair):
                            for hh in range(2):
                                h0 = 64 * hh
                                st_ps = stp.tile([128, 1024], F32, tag="st")
                                for j in range(2):
                                    ki = 2 * kp + j
                                    nc.tensor.matmul(st_ps[:, 512 * j:512 * j + 512],
                                                     ktr[h0:h0 + 64, 128 * ki:128 * ki + 128],
                                                     qtr[h0:h0 + 64, q0:q0 + 512],
                                                     start=True, stop=True)
                                p_t = pp.tile([128, 1024], BF16, tag="p")
                                nc.scalar.activation(p_t, st_ps, AF.Exp, scale=0.125)
                                if kp >= 2 * qc:
                                    # causal mask on the diagonal strip:
                                    # keep iff q0+f >= 128*(2kp+j)+p
                                    nc.gpsimd.affine_select(
                                        out=p_t, in_=p_t,
                                        compare_op=mybir.AluOpType.is_ge,
                                        fill=0.0,
                                        base=q0 - 256 * kp,
                                        pattern=[[-128, 2], [1, 512]],
                                        channel_multiplier=-1,
                                    )
                                for j in range(2):
                                    ki = 2 * kp + j
                                    col0 = 0 if hh == 0 else 64
                                    nc.tensor.matmul(pv_ps[hh],
                                                     v_sb[:, ki, col0:col0 + 128],
                                                     p_t[:, 512 * j:512 * j + 512],
                                                     start=(ki == 0), stop=(ki == 4 * qc + 3))
                        for hh in range(2):
                            data = slice(0, 64) if hh == 0 else slice(64, 128)
                            sums = slice(64, 128) if hh == 0 else slice(0, 64)
                            rc = rcp.tile([64, 512], F32, tag="rc")
                            nc.vector.reciprocal(rc, pv_ps[hh][sums, :])
                            nc.vector.tensor_mul(ot_sb[64 * hh:64 * hh + 64, q0:q0 + 512],
                                                 pv_ps[hh][data, :], rc)

                # ---- o_proj ----
                with tc.tile_pool(name=f"op{b}", bufs=4, space="PSUM") as opp:
                    for tt in range(16):
                        yo = yop.tile([128, 1024], BF16, tag="yo")
                        for oc in range(2):
                            op_ps = opp.tile([128, 512], F32, tag="op")
                            nc.tensor.matmul(op_ps, ot_sb[:, 128 * tt:128 * tt + 128],
                                             wot_sb[:, 512 * oc:512 * oc + 512],
                                             start=True, stop=True)
                            if oc == 0:
                                nc.vector.tensor_copy(yo[:, 0:512], op_ps)
                            else:
                                nc.scalar.activation(yo[:, 512:1024], op_ps, AF.Copy)
                        nc.scalar.dma_start(out=y[tb0 + 128 * tt:tb0 + 128 * tt + 128, :], in_=yo)
    nc.compile()
    return nc


_NC_CACHE = {}


def _get_nc(nb=B):
    if nb not in _NC_CACHE:
        _NC_CACHE[nb] = build_nc(nb)
    return _NC_CACHE[nb]


def _host_prep(x, Wq, Wk, Wv, Wo):
    x2 = np.ascontiguousarray(x.reshape(BS, D)).astype(BFNP)

    half = 32
    inv_freq = 1.0 / (ROPE_THETA ** (np.arange(half, dtype=np.float64) / half))
    freqs = np.arange(S, dtype=np.float64)[:, None] * inv_freq[None, :]
    c_ = np.cos(freqs).astype(np.float32).T      # [32, S]
    s_ = np.sin(freqs).astype(np.float32).T
    cos2 = np.ascontiguousarray(np.tile(c_, (4, 1)))            # [128, S]
    sins2 = np.ascontiguousarray(np.vstack([-s_, s_, -s_, s_]))  # [128, S]

    perm = np.zeros(128, dtype=np.int64)
    partner = np.zeros(128, dtype=np.int64)
    for hh in range(2):
        for j in range(64):
            perm[64 * hh + j] = 64 * hh + (2 * j if j < 32 else 2 * (j - 32) + 1)
            partner[64 * hh + j] = 64 * hh + (j + 32) % 64
    pswap = np.zeros((128, 128), dtype=np.float32)
    pswap[partner, np.arange(128)] = 1.0

    in_maps = []
    for c in range(NCORES):
        sl = slice(128 * c, 128 * c + 128)
        in_maps.append({
            "xb": x2,
            "wqt": np.ascontiguousarray(Wq[sl][perm].T).astype(BFNP),
            "wkt": np.ascontiguousarray(Wk[sl][perm].T).astype(BFNP),
            "wvt": np.ascontiguousarray(Wv[sl].T).astype(BFNP),
            "wot": np.ascontiguousarray(Wo[:, sl].T).astype(np.float32),
            "cos2": cos2,
            "sins2": sins2,
            "pswap": pswap,
        })
    return in_maps


def kernel(x, Wq, Wk, Wv, Wo):
    global LAST_RESULTS
    x = np.asarray(x, dtype=np.float32)
    Wq = np.asarray(Wq, dtype=np.float32)
    Wk = np.asarray(Wk, dtype=np.float32)
    Wv = np.asarray(Wv, dtype=np.float32)
    Wo = np.asarray(Wo, dtype=np.float32)

    nc = _get_nc(B)
    in_maps = _host_prep(x, Wq, Wk, Wv, Wo)
    res = run_bass_kernel_spmd(nc, in_maps, core_ids=list(range(NCORES)),
                               trace=TRACE)
    LAST_RESULTS = res
    out = np.zeros((BS, D), dtype=np.float32)
    for c in range(NCORES):
        out += np.asarray(res.results[c]["y"]).astype(np.float32)
    return out.reshape(B, S, D)


# revision 35
# speedup vs baseline: 73.2718x; 73.2718x over previous
"""Causal MHA with RoPE on 8 Trainium2 NeuronCores.

Sharding: tensor-parallel over heads. Core c owns heads {2c, 2c+1} (a 128-wide
slice of the model dim). Each core computes Q/K/V projections for its heads,
full causal attention, and a partial o_proj; the host sums the 8 partial
outputs (the "all-reduce").

Device layouts (per core):
  x.T   [128 i-part, 8 i-tile, t]   bf16, via DMA xbar transpose of bf16 x
  Q.T/K.T [128 hd, t] bf16, hd = [head A (ev0..31, od0..31), head B ...]
  RoPE = cos-mul + (permutation matmul on PE) sin-mul + add, tables from host
  scores S.T [k, q] = K.T-slice.T @ Q.T-slice (K=64, heads row-packed on PE)
  softmax: exp on ACT (no max subtraction; scores are O(1)), causal mask via
  gpsimd affine_select on the diagonal strip, denominators via ones-columns
  appended to V in the PV matmul; reciprocal computed on a PE-transposed
  [128, 8] layout (cheap), broadcast back via tiny PE matmuls.
  o_proj: O.T tiles (bf16) x Wo.T slice (bf16) -> partial y, written bf16.
"""
import sys
sys.path.insert(0, '/opt/trn_rl_repo')

import numpy as np
import ml_dtypes

import concourse.bass as bass
from concourse import bacc
import concourse.mybir as mybir
import concourse.tile as tile
from concourse.bass_utils import run_bass_kernel_spmd

BFNP = ml_dtypes.bfloat16
F32 = mybir.dt.float32
F32R = mybir.dt.float32r
BF16 = mybir.dt.bfloat16
AF = mybir.ActivationFunctionType

B, S, D = 4, 2048, 1024
NCORES = 8
BS = B * S
ROPE_THETA = 10000.0

TRACE = False
LAST_RESULTS = None


def build_nc(nb=B, debug=False):
    nc = bacc.Bacc()
    dbg = {}
    if debug:
        for nm in ("d_sT", "d_rT", "d_rB", "d_bc", "d_s2", "d_ot"):
            shp = [128, 16] if nm in ("d_sT", "d_rT") else ([2, 512] if nm == "d_rB" else [128, 512])
            dbg[nm] = nc.dram_tensor(nm, shp, F32, kind="ExternalOutput")
        for nm in ("d_qtr", "d_ktr", "d_p", "d_qt"):
            dbg[nm] = nc.dram_tensor(nm, [128, 1024], F32, kind="ExternalOutput")
    xb = nc.dram_tensor("xb", [BS, D], BF16, kind="ExternalInput")
    wqt = nc.dram_tensor("wqt", [D, 128], BF16, kind="ExternalInput")
    wkt = nc.dram_tensor("wkt", [D, 128], BF16, kind="ExternalInput")
    wvt = nc.dram_tensor("wvt", [D, 128], BF16, kind="ExternalInput")
    wot = nc.dram_tensor("wot", [128, D], BF16, kind="ExternalInput")
    cos2 = nc.dram_tensor("cos2", [128, S], F32, kind="ExternalInput")
    sins2 = nc.dram_tensor("sins2", [128, S], F32, kind="ExternalInput")
    pswap = nc.dram_tensor("pswap", [128, 128], F32, kind="ExternalInput")
    iden2 = nc.dram_tensor("iden2", [2, 2], F32, kind="ExternalInput")
    iden128 = nc.dram_tensor("iden128", [128, 128], F32, kind="ExternalInput")
    sel2 = nc.dram_tensor("sel2", [2, 128], F32, kind="ExternalInput")
    y = nc.dram_tensor("y", [BS, D], BF16, kind="ExternalOutput")

    with tile.TileContext(nc) as tc:
        with tc.tile_pool(name="const", bufs=1) as constp, \
             tc.tile_pool(name="xt", bufs=2) as xtp, \
             tc.tile_pool(name="qk", bufs=3) as qkp, \
             tc.tile_pool(name="vsb", bufs=2) as vsp, \
             tc.tile_pool(name="rope", bufs=4) as ropep, \
             tc.tile_pool(name="ptile", bufs=8) as pp, \
             tc.tile_pool(name="otp", bufs=3) as otp, \
             tc.tile_pool(name="rc", bufs=2) as rcp, \
             tc.tile_pool(name="yout", bufs=4) as yop:
            wq_sb = constp.tile([128, 8, 128], BF16)
            nc.scalar.dma_start(out=wq_sb, in_=wqt.rearrange("(a p) f -> p a f", p=128))
            wk_sb = constp.tile([128, 8, 128], BF16)
            nc.scalar.dma_start(out=wk_sb, in_=wkt.rearrange("(a p) f -> p a f", p=128))
            wv_sb = constp.tile([128, 8, 128], BF16)
            nc.scalar.dma_start(out=wv_sb, in_=wvt.rearrange("(a p) f -> p a f", p=128))
            wot_sb = constp.tile([128, D], BF16)
            nc.scalar.dma_start(out=wot_sb, in_=wot[:, :])
            cos_sb = constp.tile([128, S], F32)
            nc.scalar.dma_start(out=cos_sb, in_=cos2[:, :])
            sin_sb = constp.tile([128, S], F32)
            nc.scalar.dma_start(out=sin_sb, in_=sins2[:, :])
            psw_sb = constp.tile([128, 128], F32R)
            nc.scalar.dma_start(out=psw_sb, in_=pswap[:, :].bitcast(F32R))
            id2_sb = constp.tile([2, 2], F32R)
            nc.scalar.dma_start(out=id2_sb, in_=iden2[:, :].bitcast(F32R))
            id128_sb = constp.tile([128, 128], F32R)
            nc.scalar.dma_start(out=id128_sb, in_=iden128[:, :].bitcast(F32R))
            sel2_sb = constp.tile([2, 128], F32R)
            nc.scalar.dma_start(out=sel2_sb, in_=sel2[:, :].bitcast(F32R))

            for bb in range(nb):
                b = bb % B
                tb0 = b * S
                qtr = qkp.tile([128, S], BF16, tag="qtr")
                ktr = qkp.tile([128, S], BF16, tag="ktr")
                v_sb = vsp.tile([128, 16, 192], BF16, tag="v")
                nc.gpsimd.memset(v_sb[:, :, 64:128], 1.0)
                ot_sb = otp.tile([128, S], BF16, tag="ot")

                # ---- projections + rope, per 512-token chunk ----
                with tc.tile_pool(name=f"pj{b}", bufs=6, space="PSUM") as pjp, \
                     tc.tile_pool(name=f"sw{b}", bufs=2, space="PSUM") as swp:
                    for c4 in range(4):
                        t0 = 512 * c4
                        xt = xtp.tile([128, 8, 512], BF16, tag="xt")
                        nc.sync.dma_start_transpose(xt, xb[tb0 + t0:tb0 + t0 + 512, :])
                        qt_ps = pjp.tile([128, 512], F32, tag="pj")
                        kt_ps = pjp.tile([128, 512], F32, tag="pj")
                        vt_ps = pjp.tile([128, 512], F32, tag="pj")
                        for it in range(8):
                            nc.tensor.matmul(qt_ps, wq_sb[:, it, :], xt[:, it, :],
                                             start=(it == 0), stop=(it == 7))
                        for it in range(8):
                            nc.tensor.matmul(kt_ps, wk_sb[:, it, :], xt[:, it, :],
                                             start=(it == 0), stop=(it == 7))
                        for tt in range(4):
                            for it in range(8):
                                nc.tensor.matmul(vt_ps[:, 128 * tt:128 * tt + 128],
                                                 xt[:, it, 128 * tt:128 * tt + 128],
                                                 wv_sb[:, it, :],
                                                 start=(it == 0), stop=(it == 7))
                        # rope: dst = ps*cos + (pswap.T @ ps)*sins
                        for ps_t, dst in ((qt_ps, qtr), (kt_ps, ktr)):
                            sb_c = ropep.tile([128, 512], F32R, tag="rcopy")
                            nc.vector.tensor_copy(sb_c, ps_t)
                            if debug and b == 0 and c4 == 0 and ps_t is qt_ps:
                                nc.sync.dma_start(out=dbg["d_qt"][:, 0:512], in_=sb_c.bitcast(F32))
                            sw_ps = swp.tile([128, 512], F32, tag="sw")
                            nc.tensor.matmul(sw_ps, psw_sb, sb_c, start=True, stop=True)
                            tmp = ropep.tile([128, 512], F32, tag="rtmp")
                            nc.vector.tensor_mul(tmp, sw_ps, sin_sb[:, t0:t0 + 512])
                            tmp2 = ropep.tile([128, 512], F32, tag="rtmp2")
                            nc.vector.tensor_mul(tmp2, ps_t, cos_sb[:, t0:t0 + 512])
                            nc.vector.tensor_add(dst[:, t0:t0 + 512], tmp2, tmp)
                        # v: [t, hd] -> v_sb k-tiles [VA(64) | ones(64) | VB(64)]
                        vv = vt_ps.rearrange("p (t c) -> p t c", t=4)
                        nc.vector.tensor_copy(v_sb[:, 4 * c4:4 * c4 + 4, 0:64], vv[:, :, 0:64])
                        nc.vector.tensor_copy(v_sb[:, 4 * c4:4 * c4 + 4, 128:192], vv[:, :, 64:128])
                        if debug and b == 0 and c4 == 0:
                            dq = ropep.tile([128, 1024], F32, tag="dq")
                            nc.vector.tensor_copy(dq[:, 0:512], qtr[:, 0:512])
                            nc.vector.tensor_copy(dq[:, 512:1024], ktr[:, 0:512])
                            nc.sync.dma_start(out=dbg["d_qtr"][:, :], in_=dq)

                # ---- attention, per 512-query chunk; o_proj as a phase after ----
                with tc.tile_pool(name=f"st{b}", bufs=2, space="PSUM") as stp, \
                     tc.tile_pool(name=f"pv{b}", bufs=3, space="PSUM") as pvp, \
                     tc.tile_pool(name=f"nm{b}", bufs=1, space="PSUM") as nmp:
                    for qc in range(4):
                        q0 = 512 * qc
                        npair = 2 * (qc + 1)
                        pv_ps = []
                        for _hh in range(2):
                            pv_one = pvp.tile([128, 512], F32, tag="pv")
                            pv_ps.append(pv_one)
                        for kp in range(npair):
                            p_ts = {}
                            for hh in range(2):
                                h0 = 64 * hh
                                st_ps = stp.tile([128, 1024], F32, tag="st")
                                for j in range(2):
                                    ki = 2 * kp + j
                                    nc.tensor.matmul(st_ps[:, 512 * j:512 * j + 512],
                                                     ktr[h0:h0 + 64, 128 * ki:128 * ki + 128],
                                                     qtr[h0:h0 + 64, q0:q0 + 512],
                                                     start=True, stop=True)
                                p_t = pp.tile([128, 1024], BF16, tag="p")
                                nc.scalar.activation(p_t, st_ps, AF.Exp, scale=0.125)
                                if debug and b == 0 and qc == 0 and kp == 0 and hh == 0:
                                    dcp3 = pp.tile([128, 1024], F32, tag="dcp3")
                                    nc.vector.tensor_copy(dcp3, st_ps)
                                    nc.sync.dma_start(out=dbg["d_p"][:, :], in_=dcp3)
                                if kp >= 2 * qc:
                                    nc.gpsimd.affine_select(
                                        out=p_t, in_=p_t,
                                        compare_op=mybir.AluOpType.is_ge,
                                        fill=0.0,
                                        base=q0 - 256 * kp,
                                        pattern=[[-128, 2], [1, 512]],
                                        channel_multiplier=-1,
                                    )
                                p_ts[hh] = p_t
                            for hh in range(2):
                                for j in range(2):
                                    ki = 2 * kp + j
                                    col0 = 0 if hh == 0 else 64
                                    nc.tensor.matmul(pv_ps[hh],
                                                     v_sb[:, ki, col0:col0 + 128],
                                                     p_ts[hh][:, 512 * j:512 * j + 512],
                                                     start=(ki == 0), stop=(ki == 4 * qc + 3))
                        # normalize: recip of sums via PE transpose to [128, 8]
                        # pv_A: data rows 0:64, sums 64:128; pv_B: sums 0:64, data 64:128
                        s2_sb = rcp.tile([128, 512], F32R, tag="s2")
                        nc.vector.tensor_copy(s2_sb[0:64, :], pv_ps[0][64:128, :])
                        nc.vector.tensor_copy(s2_sb[64:128, :], pv_ps[1][0:64, :])
                        sT_ps = nmp.tile([128, 16], F32, tag="nm")
                        # transpose sums chunks: out[:, 4cc+2hh:+2] = dup-block.T @ [e1 e2]
                        e2_at = (id128_sb[0:64, 0:2], id128_sb[64:128, 64:66])
                        for cc in range(4):
                            for hh in range(2):
                                nc.tensor.matmul(sT_ps[:, 4 * cc + 2 * hh:4 * cc + 2 * hh + 2],
                                                 s2_sb[64 * hh:64 * hh + 64, 128 * cc:128 * cc + 128],
                                                 e2_at[hh], start=True, stop=True)
                        rT_sb = rcp.tile([128, 16], F32R, tag="rT")
                        with nc.allow_low_precision(reason="softmax denom recip in f32r"):
                            nc.vector.reciprocal(rT_sb, sT_ps)
                        rB_ps = nmp.tile([2, 512], F32, tag="nm")
                        for cc in range(4):
                            nc.tensor.matmul(rB_ps[:, 128 * cc:128 * cc + 128],
                                             rT_sb[:, 4 * cc:4 * cc + 3:2],
                                             id128_sb, start=True, stop=True)
                        rB_sb = rcp.tile([2, 512], F32R, tag="rB")
                        nc.vector.tensor_copy(rB_sb, rB_ps)
                        bc_ps = nmp.tile([128, 512], F32, tag="nm")
                        nc.tensor.matmul(bc_ps, sel2_sb, rB_sb, start=True, stop=True)
                        bc_sb = rcp.tile([128, 512], F32, tag="bc")
                        nc.vector.tensor_copy(bc_sb, bc_ps)
                        if debug and b == 0 and qc == 0:
                            dcp = rcp.tile([128, 512], F32, tag="dcp")
                            nc.vector.tensor_copy(dcp[:, 0:16], sT_ps)
                            nc.sync.dma_start(out=dbg["d_sT"][:, :], in_=dcp[:, 0:16])
                            nc.sync.dma_start(out=dbg["d_rT"][:, :], in_=rT_sb.bitcast(F32))
                            nc.sync.dma_start(out=dbg["d_rB"][:, :], in_=rB_sb.bitcast(F32))
                            nc.sync.dma_start(out=dbg["d_bc"][:, :], in_=bc_sb)
                            nc.sync.dma_start(out=dbg["d_s2"][:, :], in_=s2_sb.bitcast(F32))
                            dcp2 = rcp.tile([128, 512], F32, tag="dcp")
                            nc.vector.tensor_copy(dcp2[0:64, :], pv_ps[0][0:64, :])
                            nc.vector.tensor_copy(dcp2[64:128, :], pv_ps[1][64:128, :])
                            nc.sync.dma_start(out=dbg["d_ot"][:, :], in_=dcp2)
                        nc.vector.tensor_mul(ot_sb[0:64, q0:q0 + 512],
                                             pv_ps[0][0:64, :], bc_sb[0:64, :])
                        nc.vector.tensor_mul(ot_sb[64:128, q0:q0 + 512],
                                             pv_ps[1][64:128, :], bc_sb[64:128, :])

                # ---- o_proj ----
                with tc.tile_pool(name=f"op{b}", bufs=4, space="PSUM") as opp:
                    for tt in range(16):
                        yo = yop.tile([128, 1024], BF16, tag="yo")
                        for oc in range(2):
                            op_ps = opp.tile([128, 512], F32, tag="op")
                            nc.tensor.matmul(op_ps, ot_sb[:, 128 * tt:128 * tt + 128],
                                             wot_sb[:, 512 * oc:512 * oc + 512],
                                             start=True, stop=True)
                            if oc == 0:
                                nc.vector.tensor_copy(yo[:, 0:512], op_ps)
                            else:
                                nc.scalar.activation(yo[:, 512:1024], op_ps, AF.Copy)
                        nc.scalar.dma_start(out=y[tb0 + 128 * tt:tb0 + 128 * tt + 128, :], in_=yo)

    nc.compile()
    return nc


_NC_CACHE = {}


def _get_nc(nb=B):
    if nb not in _NC_CACHE:
        _NC_CACHE[nb] = build_nc(nb)
    return _NC_CACHE[nb]


def _host_prep(x, Wq, Wk, Wv, Wo):
    x2 = np.ascontiguousarray(x.reshape(BS, D)).astype(BFNP)

    half = 32
    inv_freq = 1.0 / (ROPE_THETA ** (np.arange(half, dtype=np.float64) / half))
    freqs = np.arange(S, dtype=np.float64)[:, None] * inv_freq[None, :]
    c_ = np.cos(freqs).astype(np.float32).T      # [32, S]
    s_ = np.sin(freqs).astype(np.float32).T
    cos2 = np.ascontiguousarray(np.tile(c_, (4, 1)))            # [128, S]
    sins2 = np.ascontiguousarray(np.vstack([-s_, s_, -s_, s_]))  # [128, S]

    perm = np.zeros(128, dtype=np.int64)
    partner = np.zeros(128, dtype=np.int64)
    for hh in range(2):
        for j in range(64):
            perm[64 * hh + j] = 64 * hh + (2 * j if j < 32 else 2 * (j - 32) + 1)
            partner[64 * hh + j] = 64 * hh + (j + 32) % 64
    pswap = np.zeros((128, 128), dtype=np.float32)
    pswap[partner, np.arange(128)] = 1.0

    iden2 = np.eye(2, dtype=np.float32)
    iden128 = np.eye(128, dtype=np.float32)
    sel2 = np.zeros((2, 128), dtype=np.float32)
    sel2[0, 0:64] = 1.0
    sel2[1, 64:128] = 1.0

    in_maps = []
    for c in range(NCORES):
        sl = slice(128 * c, 128 * c + 128)
        in_maps.append({
            "xb": x2,
            "wqt": np.ascontiguousarray(Wq[sl][perm].T).astype(BFNP),
            "wkt": np.ascontiguousarray(Wk[sl][perm].T).astype(BFNP),
            "wvt": np.ascontiguousarray(Wv[sl].T).astype(BFNP),
            "wot": np.ascontiguousarray(Wo[:, sl].T).astype(BFNP),
            "cos2": cos2,
            "sins2": sins2,
            "pswap": pswap,
            "iden2": iden2,
            "iden128": iden128,
            "sel2": sel2,
        })
    return in_maps


def kernel(x, Wq, Wk, Wv, Wo):
    global LAST_RESULTS
    x = np.asarray(x, dtype=np.float32)
    Wq = np.asarray(Wq, dtype=np.float32)
    Wk = np.asarray(Wk, dtype=np.float32)
    Wv = np.asarray(Wv, dtype=np.float32)
    Wo = np.asarray(Wo, dtype=np.float32)

    nc = _get_nc(B)
    in_maps = _host_prep(x, Wq, Wk, Wv, Wo)
    res = run_bass_kernel_spmd(nc, in_maps, core_ids=list(range(NCORES)),
                               trace=TRACE)
    LAST_RESULTS = res
    out = np.zeros((BS, D), dtype=np.float32)
    for c in range(NCORES):
        out += np.asarray(res.results[c]["y"]).astype(np.float32)
    return out.reshape(B, S, D)
